# revision 8
# baseline (speedup 1.0000x reference)
"""Trainium2 Bass kernel: causal MultiHeadAttention with RoPE.

B=1, S=4096, D=768, H=12 heads, dk=64, fp32 I/O. 8 NeuronCores, SPMD.

Sharding: snake-interleaved query tiles. Core c owns the two 256-row query
tiles {c, 8+c} (of 16), which balances causal attention work exactly. Every
core redundantly computes the full K and V projections (cheap vs. any
collective), computes flash-style attention for its 512 query rows over all
12 heads, applies the output projection for those rows, and writes its
[512, 768] slice. The host scatters slices into the full output.

Device-side layouts (all produced by host-side prep, no device transposes):
  - xt:  X^T [768, 4096] bf16, k-tile columns permuted per-core (sigma) so
         the attention loop's k-iteration order is static & uniform.
  - wq/wk/wv/wo: W^T [768, 768] bf16 (matmul contraction on partitions).
  - cos/sin tables for RoPE in the [d, s] layout (pair-swap via
    stream_shuffle; sign baked into the sin table).
  - causal handling: two static triangle masks for the diagonal k-tiles
    (always iterations 0,1 of each slot) + per-pair exp bias (-100 kills
    padded tiles) supplied as data, keeping one identical program per core.
  - softmax denominators via a ones-column appended to V (row 64 of the
    PV accumulator); per-head normalization with DVE reciprocal + GpSimd
    partition_broadcast; normalized attention lands directly in the
    o_proj stationary layout.
"""

import sys

if "/opt/trn_rl_repo" not in sys.path:
    sys.path.insert(0, "/opt/trn_rl_repo")

import numpy as np
import ml_dtypes

D_MODEL = 768
H = 12
DK = 64
S = 4096
THETA = 10000.0
MAX_SEQ_LEN = 4096
N_CORES = 8
QT = 256            # query rows per slot
N_KT = S // 128     # 32 k-tiles of 128
EB = D_MODEL // 128  # 6 e/d blocks of 128
N_CH = S // 512     # 8 projection chunks of 512
VW = H * 65         # V_aug row width per s-tile (12 heads x (64+ones))

BF16 = ml_dtypes.bfloat16

# Iteration -> storage-slot maps (identical on every core; per-core variation
# is entirely in the data: sigma-permuted xt/cos/sin, bias tables).
IT0_MAP = [0, 1] + list(range(4, 18))            # slot0: 16 iterations
IT1_MAP = [2, 3, 0, 1] + list(range(4, 32))      # slot1: 32 iterations


def _sigma(c):
    """Storage permutation: which k-tile sits in storage slot i for core c."""
    specials = [2 * c, 2 * c + 1, 2 * c + 16, 2 * c + 17]
    rest = [t for t in range(N_KT) if t not in specials]
    return specials + rest


def _bias_cols(c):
    """Per-exp-pair bias: 0.0 keeps the pair of k-tiles, -100 kills it."""
    sig = _sigma(c)
    cols = []
    # slot0 (q-tile T=c, live k-tiles [0, 2c+2)): 8 pairs
    for p in range(8):
        if p == 0:
            cols.append(0.0)  # diagonal pair, masked
        else:
            tid = sig[IT0_MAP[2 * p]]
            cols.append(0.0 if tid <= 2 * c - 1 else -100.0)
    # slot1 (q-tile T=8+c, live k-tiles [0, 2c+18)): 16 pairs
    for p in range(16):
        if p == 0:
            cols.append(0.0)  # diagonal pair
        elif p == 1:
            cols.append(0.0)  # storage 0,1 = tiles 2c,2c+1, always live
        else:
            tid = sig[IT1_MAP[2 * p]]
            cols.append(0.0 if tid < 2 * c + 16 else -100.0)
    return np.asarray(cols, np.float32)


def build_program():
    import concourse.mybir as mybir
    import concourse.tile as tile
    from concourse import bacc, library_config

    f32 = mybir.dt.float32
    bf16 = mybir.dt.bfloat16
    Exp = mybir.ActivationFunctionType.Exp
    Copy = mybir.ActivationFunctionType.Copy

    nc = bacc.Bacc(
        "TRN2",
        target_bir_lowering=False,
        debug=False,
        enable_asserts=True,
        num_devices=N_CORES,
    )

    xt_d = nc.dram_tensor("xt", [D_MODEL, S], bf16, kind="ExternalInput")
    xtq_d = nc.dram_tensor("xtq", [D_MODEL, 2 * QT], bf16, kind="ExternalInput")
    w_d = {
        n: nc.dram_tensor(n, [D_MODEL, D_MODEL], bf16, kind="ExternalInput")
        for n in ("wq", "wk", "wv", "wo")
    }
    cosk_d = nc.dram_tensor("cosk", [128, S], bf16, kind="ExternalInput")
    sink_d = nc.dram_tensor("sink", [128, S], bf16, kind="ExternalInput")
    cosq_d = nc.dram_tensor("cosq", [128, 2 * QT], bf16, kind="ExternalInput")
    sinq_d = nc.dram_tensor("sinq", [128, 2 * QT], bf16, kind="ExternalInput")
    mask_d = nc.dram_tensor("maskab", [128, 512], bf16, kind="ExternalInput")
    bias_d = nc.dram_tensor("biasp", [128, 24], f32, kind="ExternalInput")
    out_d = nc.dram_tensor("out", [2 * QT, D_MODEL], f32, kind="ExternalOutput")

    PAIRSWAP = [i ^ 1 for i in range(32)]

    with tile.TileContext(nc) as tc:
        with (
            tc.tile_pool(name="const", bufs=1) as cpool,
            tc.tile_pool(name="wp", bufs=2) as wpool,
            tc.tile_pool(name="wop", bufs=2) as wopool,
            tc.tile_pool(name="cs", bufs=2) as cspool,
            tc.tile_pool(name="xtp", bufs=2) as xtpool,
            tc.tile_pool(name="rope", bufs=3) as rpool,
            tc.tile_pool(name="expp", bufs=4) as epool,
            tc.tile_pool(name="norm", bufs=2) as npool,
            tc.tile_pool(name="outp", bufs=2) as opool,
            tc.tile_pool(name="ps_proj", bufs=2, space="PSUM") as psb,
            tc.tile_pool(name="ps_sc", bufs=3, space="PSUM") as pssc,
            tc.tile_pool(name="ps_small", bufs=1, space="PSUM") as pss,
            tc.tile_pool(name="ps_pv", bufs=2, space="PSUM") as psv,
        ):
            nc.gpsimd.load_library(library_config.attn)

            # ---- persistent tensors (allocated once) ----
            def load_w(n):
                t = wpool.tile([128, EB * D_MODEL], bf16, tag="w")
                for eb in range(EB):
                    nc.sync.dma_start(
                        out=t[:, eb * D_MODEL:(eb + 1) * D_MODEL],
                        in_=w_d[n][eb * 128:(eb + 1) * 128, :],
                    )
                return t
            cosq = cpool.tile([128, 2 * QT], bf16, tag="cosq")
            nc.sync.dma_start(out=cosq[:], in_=cosq_d[:])
            sinq = cpool.tile([128, 2 * QT], bf16, tag="sinq")
            nc.sync.dma_start(out=sinq[:], in_=sinq_d[:])
            maskab = cpool.tile([128, 512], bf16, tag="maskab")
            nc.sync.dma_start(out=maskab[:], in_=mask_d[:])
            biasp = cpool.tile([128, 24], f32, tag="biasp")
            nc.sync.dma_start(out=biasp[:], in_=bias_d[:])
            xtq = cpool.tile([128, EB * 2 * QT], bf16, tag="xtq")
            for eb in range(EB):
                nc.sync.dma_start(
                    out=xtq[:, eb * 2 * QT:(eb + 1) * 2 * QT],
                    in_=xtq_d[eb * 128:(eb + 1) * 128, :],
                )

            kt = cpool.tile([128, EB * S], bf16, tag="kt")          # K^T, RoPE'd
            vaug = cpool.tile([128, N_KT * VW], bf16, tag="vaug")   # V + ones col
            qt = cpool.tile([128, EB * 2 * QT], bf16, tag="qt")     # Q^T, RoPE'd
            attn = cpool.tile([64, H * 2 * QT], bf16, tag="attn")   # per-head out

            def rope(dst, src_ps, cos_ap, sin_ap, width):
                """dst(bf16) = rope(src_ps fp32 psum) in [d, s] layout."""
                xb = rpool.tile([128, width], bf16, tag="rope_x")
                nc.scalar.activation(xb[:], src_ps[:], Copy)
                sh = rpool.tile([128, width], bf16, tag="rope_sh")
                nc.vector.stream_shuffle(sh[:], xb[:], PAIRSWAP)
                nc.vector.tensor_mul(xb[:], xb[:], cos_ap)
                nc.vector.tensor_mul(sh[:], sh[:], sin_ap)
                nc.vector.tensor_add(dst, xb[:], sh[:])

            # ---- Q projection + RoPE ----
            wq_sb = load_w("wq")
            for db in range(EB):
                ps = psb.tile([128, 512], f32, tag="ps_proj")
                for eb in range(EB):
                    nc.tensor.matmul(
                        ps[:],
                        wq_sb[:, eb * D_MODEL + db * 128:eb * D_MODEL + db * 128 + 128],
                        xtq[:, eb * 2 * QT:(eb + 1) * 2 * QT],
                        start=(eb == 0),
                        stop=(eb == EB - 1),
                    )
                rope(qt[:, db * 2 * QT:(db + 1) * 2 * QT], ps, cosq[:], sinq[:], 512)

            # ---- K / V projections per 512-column chunk ----
            wk_sb = load_w("wk")
            wv_sb = load_w("wv")
            for ch in range(N_CH):
                xt_t = xtpool.tile([128, EB * 512], bf16, tag="xt_t")
                for eb in range(EB):
                    nc.sync.dma_start(
                        out=xt_t[:, eb * 512:(eb + 1) * 512],
                        in_=xt_d[eb * 128:(eb + 1) * 128, ch * 512:(ch + 1) * 512],
                    )
                ck = cspool.tile([128, 512], bf16, tag="cosk")
                nc.sync.dma_start(out=ck[:], in_=cosk_d[:, ch * 512:(ch + 1) * 512])
                sk = cspool.tile([128, 512], bf16, tag="sink")
                nc.sync.dma_start(out=sk[:], in_=sink_d[:, ch * 512:(ch + 1) * 512])
                # K^T blocks with RoPE
                for db in range(EB):
                    ps = psb.tile([128, 512], f32, tag="ps_proj")
                    for eb in range(EB):
                        nc.tensor.matmul(
                            ps[:],
                            wk_sb[:, eb * D_MODEL + db * 128:eb * D_MODEL + db * 128 + 128],
                            xt_t[:, eb * 512:(eb + 1) * 512],
                            start=(eb == 0),
                            stop=(eb == EB - 1),
                        )
                    rope(
                        kt[:, db * S + ch * 512:db * S + (ch + 1) * 512],
                        ps,
                        ck[:],
                        sk[:],
                        512,
                    )
                # V natural layout, interleaved ones column
                for stl in range(4):
                    st = ch * 4 + stl
                    psa = psb.tile([128, 512], f32, tag="ps_proj")
                    psb2 = pss.tile([128, 256], f32, tag="ps_vb")
                    for eb in range(EB):
                        nc.tensor.matmul(
                            psa[:],
                            xt_t[:, eb * 512 + stl * 128:eb * 512 + stl * 128 + 128],
                            wv_sb[:, eb * D_MODEL:eb * D_MODEL + 512],
                            start=(eb == 0),
                            stop=(eb == EB - 1),
                        )
                    for eb in range(EB):
                        nc.tensor.matmul(
                            psb2[:],
                            xt_t[:, eb * 512 + stl * 128:eb * 512 + stl * 128 + 128],
                            wv_sb[:, eb * D_MODEL + 512:eb * D_MODEL + 768],
                            start=(eb == 0),
                            stop=(eb == EB - 1),
                        )
                    base = st * VW
                    vtile = vaug[:, base:base + VW].rearrange(
                        "p (h d) -> p h d", d=65
                    )
                    nc.vector.memset(vtile[:, :, 64:65], 1.0)
                    nc.vector.tensor_copy(
                        vtile[:, 0:8, 0:64],
                        psa[:].rearrange("p (h d) -> p h d", d=64),
                    )
                    nc.vector.tensor_copy(
                        vtile[:, 8:12, 0:64],
                        psb2[:].rearrange("p (h d) -> p h d", d=64),
                    )

            # ---- attention ----
            for h in range(H):
                kb = h // 2
                ro = 64 * (h % 2)
                for s in range(2):
                    n_pairs = 8 if s == 0 else 16
                    itmap = IT0_MAP if s == 0 else IT1_MAP
                    bias_off = 0 if s == 0 else 8
                    pv = psv.tile([65, QT], f32, tag="ps_pv")
                    for p in range(n_pairs):
                        i0 = itmap[2 * p]
                        i1 = itmap[2 * p + 1]
                        sc = pssc.tile([128, 512], f32, tag="ps_sc")
                        nc.tensor.matmul(
                            sc[:, 0:256],
                            kt[ro:ro + 64, kb * S + i0 * 128:kb * S + i0 * 128 + 128],
                            qt[ro:ro + 64, kb * 2 * QT + s * QT:kb * 2 * QT + s * QT + QT],
                            start=True,
                            stop=True,
                        )
                        nc.tensor.matmul(
                            sc[:, 256:512],
                            kt[ro:ro + 64, kb * S + i1 * 128:kb * S + i1 * 128 + 128],
                            qt[ro:ro + 64, kb * 2 * QT + s * QT:kb * 2 * QT + s * QT + QT],
                            start=True,
                            stop=True,
                        )
                        et = epool.tile([128, 512], bf16, tag="et")
                        nc.scalar.activation(
                            et[:],
                            sc[:],
                            Exp,
                            bias=biasp[:, bias_off + p:bias_off + p + 1],
                            scale=0.125,
                        )
                        if p == 0:
                            nc.vector.tensor_mul(et[:], et[:], maskab[:])
                        nc.tensor.matmul(
                            pv[:],
                            vaug[:, i0 * VW + h * 65:i0 * VW + h * 65 + 65],
                            et[:, 0:256],
                            start=(p == 0),
                            stop=False,
                        )
                        nc.tensor.matmul(
                            pv[:],
                            vaug[:, i1 * VW + h * 65:i1 * VW + h * 65 + 65],
                            et[:, 256:512],
                            start=False,
                            stop=(p == n_pairs - 1),
                        )
                    # normalize: recip of sums (row 64) -> physical row 0
                    # (partition_broadcast reads the tile's partition 0),
                    # broadcast, multiply
                    rc = npool.tile([128, QT], f32, tag="recip")
                    nc.vector.reciprocal(rc[64:65, :], pv[64:65, :])
                    r0 = npool.tile([1, QT], f32, tag="r0")
                    nc.sync.dma_start(out=r0[:], in_=rc[64:65, :])
                    rb = npool.tile([128, QT], f32, tag="rbcast")
                    nc.gpsimd.partition_broadcast(rb[0:64, :], r0[0:1, :])
                    nc.vector.tensor_mul(
                        attn[0:64, h * 2 * QT + s * QT:h * 2 * QT + s * QT + QT],
                        pv[0:64, :],
                        rb[0:64, :],
                    )

            # ---- output projection (contraction over heads, K=64 each) ----
            for qtl in range(4):
                po1 = pssc.tile([128, 512], f32, tag="ps_sc")
                po2 = pss.tile([128, 256], f32, tag="ps_vb")
                for h in range(H):
                    lhs = attn[0:64, h * 2 * QT + qtl * 128:h * 2 * QT + qtl * 128 + 128]
                    wo_t = wopool.tile([64, D_MODEL], bf16, tag="wo_t")
                    nc.sync.dma_start(
                        out=wo_t[:], in_=w_d["wo"][h * 64:(h + 1) * 64, :]
                    )
                    nc.tensor.matmul(
                        po1[:],
                        lhs,
                        wo_t[:, 0:512],
                        start=(h == 0),
                        stop=(h == H - 1),
                    )
                    nc.tensor.matmul(
                        po2[:],
                        lhs,
                        wo_t[:, 512:768],
                        start=(h == 0),
                        stop=(h == H - 1),
                    )
                osb = opool.tile([128, D_MODEL], f32, tag="osb")
                nc.vector.tensor_copy(osb[:, 0:512], po1[:])
                nc.vector.tensor_copy(osb[:, 512:768], po2[:])
                nc.sync.dma_start(
                    out=out_d[qtl * 128:(qtl + 1) * 128, :], in_=osb[:]
                )

    nc.compile()
    return nc


_PROGRAM = None


def _get_program():
    global _PROGRAM
    if _PROGRAM is None:
        _PROGRAM = build_program()
    return _PROGRAM


def host_prep(in_features, token_positions, q_proj, k_proj, v_proj, o_proj):
    """Build the 8 per-core input maps."""
    x = np.asarray(in_features, np.float32).reshape(S, D_MODEL)
    tp = np.asarray(token_positions)
    qp = np.asarray(q_proj, np.float32)
    kp = np.asarray(k_proj, np.float32)
    vp = np.asarray(v_proj, np.float32)
    op = np.asarray(o_proj, np.float32)

    xt = np.ascontiguousarray(x.T)                      # [768, 4096] fp32
    xt_bf = xt.astype(BF16)
    wq = np.ascontiguousarray(qp.T).astype(BF16)
    wk = np.ascontiguousarray(kp.T).astype(BF16)
    wv = np.ascontiguousarray(vp.T).astype(BF16)
    wo = np.ascontiguousarray(op.T).astype(BF16)

    inv_freq = 1.0 / THETA ** (np.arange(0, DK, 2, dtype=np.float32) / DK)
    pos = np.clip(tp.astype(np.float32), 0, MAX_SEQ_LEN - 1)
    freq = pos[:, None] * inv_freq[None, :]             # [S, 32]
    cos_t, sin_t = np.cos(freq), np.sin(freq)

    r = np.arange(128)
    fidx = (r % 64) // 2
    sign = np.where(r % 2 == 0, -1.0, 1.0).astype(np.float32)
    cos128 = cos_t[:, fidx].T.astype(np.float32)        # [128, S]
    sin128 = (sin_t[:, fidx].T * sign[:, None]).astype(np.float32)

    # diagonal masks: A = k-tile aligned with q[0:256) first half,
    # B = aligned with second half. scores^T layout: [k(128), q(256)].
    ki = np.arange(128)[:, None]
    qi = np.arange(QT)[None, :]
    mask_a = (ki <= qi).astype(np.float32)
    mask_b = (ki + 128 <= qi).astype(np.float32)
    maskab = np.concatenate([mask_a, mask_b], axis=1).astype(BF16)

    in_maps = []
    for c in range(N_CORES):
        sig = _sigma(c)
        perm = np.concatenate(
            [np.arange(t * 128, (t + 1) * 128) for t in sig]
        )
        qcols = np.concatenate(
            [
                np.arange(QT * c, QT * (c + 1)),
                np.arange(QT * (8 + c), QT * (9 + c)),
            ]
        )
        biasp = np.broadcast_to(_bias_cols(c)[None, :], (128, 24))
        in_maps.append(
            {
                "xt": np.ascontiguousarray(xt_bf[:, perm]),
                "xtq": np.ascontiguousarray(xt_bf[:, qcols]),
                "wq": wq,
                "wk": wk,
                "wv": wv,
                "wo": wo,
                "cosk": np.ascontiguousarray(cos128[:, perm]).astype(BF16),
                "sink": np.ascontiguousarray(sin128[:, perm]).astype(BF16),
                "cosq": np.ascontiguousarray(cos128[:, qcols]).astype(BF16),
                "sinq": np.ascontiguousarray(sin128[:, qcols]).astype(BF16),
                "maskab": maskab,
                "biasp": np.ascontiguousarray(biasp, np.float32),
            }
        )
    return in_maps


def assemble_output(results):
    out = np.empty((1, S, D_MODEL), np.float32)
    for c in range(N_CORES):
        r = np.asarray(results[c]["out"], np.float32)
        out[0, QT * c:QT * (c + 1)] = r[0:QT]
        out[0, QT * (8 + c):QT * (9 + c)] = r[QT:2 * QT]
    return out


def kernel(**inputs):
    from concourse.bass_utils import run_bass_kernel_spmd

    nc = _get_program()
    in_maps = host_prep(**inputs)
    res = run_bass_kernel_spmd(nc, in_maps, list(range(N_CORES)))
    return assemble_output(res.results)


if __name__ == "__main__":
    nc = build_program()
    print("program built and compiled")


# revision 11
# speedup vs baseline: 32.2063x; 32.2063x over previous
"""Trainium2 Bass kernel: causal MultiHeadAttention with RoPE.

B=1, S=4096, D=768, H=12 heads, dk=64, fp32 I/O. 8 NeuronCores, SPMD.

Sharding: snake-interleaved query tiles. Core c owns the two 256-row query
tiles {c, 8+c} (of 16), which balances causal attention work exactly. Every
core redundantly computes the full K and V projections (cheap vs. any
collective), computes flash-style attention for its 512 query rows over all
12 heads, applies the output projection for those rows, and writes its
[512, 768] slice. The host scatters slices into the full output.

Device-side layouts (all produced by host-side prep, no device transposes):
  - xt:  X^T [768, 4096] bf16, k-tile columns permuted per-core (sigma) so
         the attention loop's k-iteration order is static & uniform.
  - wq/wk/wv/wo: W^T [768, 768] bf16 (matmul contraction on partitions).
  - cos/sin tables for RoPE in the [d, s] layout (pair-swap via
    stream_shuffle; sign baked into the sin table).
  - causal handling: two static triangle masks for the diagonal k-tiles
    (always iterations 0,1 of each slot) + per-pair exp bias (-100 kills
    padded tiles) supplied as data, keeping one identical program per core.
  - softmax denominators via a ones-column appended to V (row 64 of the
    PV accumulator); per-head normalization with DVE reciprocal + GpSimd
    partition_broadcast; normalized attention lands directly in the
    o_proj stationary layout.
"""

import sys

if "/opt/trn_rl_repo" not in sys.path:
    sys.path.insert(0, "/opt/trn_rl_repo")

import numpy as np
import ml_dtypes

D_MODEL = 768
H = 12
DK = 64
S = 4096
THETA = 10000.0
MAX_SEQ_LEN = 4096
N_CORES = 8
QT = 256            # query rows per slot
N_KT = S // 128     # 32 k-tiles of 128
EB = D_MODEL // 128  # 6 e/d blocks of 128
N_CH = S // 512     # 8 projection chunks of 512
VW = H * 65         # V_aug row width per s-tile (12 heads x (64+ones))

BF16 = ml_dtypes.bfloat16

# Iteration -> storage-slot maps (identical on every core; per-core variation
# is entirely in the data: sigma-permuted xt/cos/sin, bias tables).
IT0_MAP = [0, 1] + list(range(4, 18))            # slot0: 16 iterations
IT1_MAP = [2, 3, 0, 1] + list(range(4, 32))      # slot1: 32 iterations


def _sigma(c):
    """Storage permutation: which k-tile sits in storage slot i for core c."""
    specials = [2 * c, 2 * c + 1, 2 * c + 16, 2 * c + 17]
    rest = [t for t in range(N_KT) if t not in specials]
    return specials + rest


def _bias_cols(c):
    """Per-exp-pair bias: 0.0 keeps the pair of k-tiles, -100 kills it."""
    sig = _sigma(c)
    cols = []
    # slot0 (q-tile T=c, live k-tiles [0, 2c+2)): 8 pairs
    for p in range(8):
        if p == 0:
            cols.append(0.0)  # diagonal pair, masked
        else:
            tid = sig[IT0_MAP[2 * p]]
            cols.append(0.0 if tid <= 2 * c - 1 else -100.0)
    # slot1 (q-tile T=8+c, live k-tiles [0, 2c+18)): 16 pairs
    for p in range(16):
        if p == 0:
            cols.append(0.0)  # diagonal pair
        elif p == 1:
            cols.append(0.0)  # storage 0,1 = tiles 2c,2c+1, always live
        else:
            tid = sig[IT1_MAP[2 * p]]
            cols.append(0.0 if tid < 2 * c + 16 else -100.0)
    return np.asarray(cols, np.float32)


def build_program():
    import concourse.mybir as mybir
    import concourse.tile as tile
    from concourse import bacc, library_config

    f32 = mybir.dt.float32
    bf16 = mybir.dt.bfloat16
    Exp = mybir.ActivationFunctionType.Exp
    Copy = mybir.ActivationFunctionType.Copy

    nc = bacc.Bacc(
        "TRN2",
        target_bir_lowering=False,
        debug=False,
        enable_asserts=True,
        num_devices=N_CORES,
    )

    xt_d = nc.dram_tensor("xt", [D_MODEL, S], bf16, kind="ExternalInput")
    xtq_d = nc.dram_tensor("xtq", [D_MODEL, 2 * QT], bf16, kind="ExternalInput")
    w_d = {
        n: nc.dram_tensor(n, [D_MODEL, D_MODEL], bf16, kind="ExternalInput")
        for n in ("wq", "wk", "wv", "wo")
    }
    cosk_d = nc.dram_tensor("cosk", [128, S], bf16, kind="ExternalInput")
    sink_d = nc.dram_tensor("sink", [128, S], bf16, kind="ExternalInput")
    cosq_d = nc.dram_tensor("cosq", [128, 2 * QT], bf16, kind="ExternalInput")
    sinq_d = nc.dram_tensor("sinq", [128, 2 * QT], bf16, kind="ExternalInput")
    mask_d = nc.dram_tensor("maskab", [128, 512], bf16, kind="ExternalInput")
    bias_d = nc.dram_tensor("biasp", [128, 24], f32, kind="ExternalInput")
    out_d = nc.dram_tensor("out", [2 * QT, D_MODEL], f32, kind="ExternalOutput")

    PAIRSWAP = [i ^ 1 for i in range(32)]

    with tile.TileContext(nc) as tc:
        with (
            tc.tile_pool(name="const", bufs=1) as cpool,
            tc.tile_pool(name="wp", bufs=2) as wpool,
            tc.tile_pool(name="cs", bufs=2) as cspool,
            tc.tile_pool(name="xtp", bufs=2) as xtpool,
            tc.tile_pool(name="rope", bufs=3) as rpool,
            tc.tile_pool(name="expp", bufs=4) as epool,
            tc.tile_pool(name="norm", bufs=2) as npool,
            tc.tile_pool(name="outp", bufs=2) as opool,
            tc.tile_pool(name="ps_proj", bufs=2, space="PSUM") as psb,
            tc.tile_pool(name="ps_sc", bufs=3, space="PSUM") as pssc,
            tc.tile_pool(name="ps_small", bufs=1, space="PSUM") as pss,
            tc.tile_pool(name="ps_pv", bufs=2, space="PSUM") as psv,
        ):
            nc.gpsimd.load_library(library_config.attn)

            # ---- persistent tensors (allocated once) ----
            def load_w(n):
                t = wpool.tile([128, EB * D_MODEL], bf16, tag="w")
                for eb in range(EB):
                    nc.sync.dma_start(
                        out=t[:, eb * D_MODEL:(eb + 1) * D_MODEL],
                        in_=w_d[n][eb * 128:(eb + 1) * 128, :],
                    )
                return t
            cosq = cpool.tile([128, 2 * QT], bf16, tag="cosq")
            nc.sync.dma_start(out=cosq[:], in_=cosq_d[:])
            sinq = cpool.tile([128, 2 * QT], bf16, tag="sinq")
            nc.sync.dma_start(out=sinq[:], in_=sinq_d[:])
            maskab = cpool.tile([128, 512], bf16, tag="maskab")
            nc.sync.dma_start(out=maskab[:], in_=mask_d[:])
            biasp = cpool.tile([128, 24], f32, tag="biasp")
            nc.sync.dma_start(out=biasp[:], in_=bias_d[:])
            xtq = cpool.tile([128, EB * 2 * QT], bf16, tag="xtq")
            for eb in range(EB):
                nc.sync.dma_start(
                    out=xtq[:, eb * 2 * QT:(eb + 1) * 2 * QT],
                    in_=xtq_d[eb * 128:(eb + 1) * 128, :],
                )

            # K^T and V_aug split per chunk so attention iterations only
            # depend on the chunk that produced their k-tiles
            ktc = [cpool.tile([128, EB * 512], bf16, tag=f"kt{ch}", name=f"kt{ch}") for ch in range(N_CH)]
            vc = [cpool.tile([128, 4 * VW], bf16, tag=f"va{ch}", name=f"va{ch}") for ch in range(N_CH)]
            qt = cpool.tile([128, EB * 2 * QT], bf16, tag="qt")     # Q^T, RoPE'd
            attn = cpool.tile([64, H * 2 * QT], bf16, tag="attn")   # per-head out

            def rope(dst, src_ps, cos_ap, sin_ap, width):
                """dst(bf16) = rope(src_ps fp32 psum) in [d, s] layout."""
                xb = rpool.tile([128, width], bf16, tag="rope_x")
                nc.scalar.activation(xb[:], src_ps[:], Copy)
                sh = rpool.tile([128, width], bf16, tag="rope_sh")
                nc.vector.stream_shuffle(sh[:], xb[:], PAIRSWAP)
                nc.vector.tensor_mul(xb[:], xb[:], cos_ap)
                nc.vector.tensor_mul(sh[:], sh[:], sin_ap)
                nc.vector.tensor_add(dst, xb[:], sh[:])

            # ---- Q projection + RoPE ----
            wq_sb = load_w("wq")
            for db in range(EB):
                ps = psb.tile([128, 512], f32, tag="ps_proj")
                for eb in range(EB):
                    nc.tensor.matmul(
                        ps[:],
                        wq_sb[:, eb * D_MODEL + db * 128:eb * D_MODEL + db * 128 + 128],
                        xtq[:, eb * 2 * QT:(eb + 1) * 2 * QT],
                        start=(eb == 0),
                        stop=(eb == EB - 1),
                    )
                rope(qt[:, db * 2 * QT:(db + 1) * 2 * QT], ps, cosq[:], sinq[:], 512)

            # ---- K / V projections per 512-column chunk ----
            wk_sb = load_w("wk")
            wv_sb = load_w("wv")
            for ch in range(N_CH):
                xt_t = xtpool.tile([128, EB * 512], bf16, tag="xt_t")
                for eb in range(EB):
                    nc.sync.dma_start(
                        out=xt_t[:, eb * 512:(eb + 1) * 512],
                        in_=xt_d[eb * 128:(eb + 1) * 128, ch * 512:(ch + 1) * 512],
                    )
                ck = cspool.tile([128, 512], bf16, tag="cosk")
                nc.sync.dma_start(out=ck[:], in_=cosk_d[:, ch * 512:(ch + 1) * 512])
                sk = cspool.tile([128, 512], bf16, tag="sink")
                nc.sync.dma_start(out=sk[:], in_=sink_d[:, ch * 512:(ch + 1) * 512])
                # K^T blocks with RoPE
                for db in range(EB):
                    ps = psb.tile([128, 512], f32, tag="ps_proj")
                    for eb in range(EB):
                        nc.tensor.matmul(
                            ps[:],
                            wk_sb[:, eb * D_MODEL + db * 128:eb * D_MODEL + db * 128 + 128],
                            xt_t[:, eb * 512:(eb + 1) * 512],
                            start=(eb == 0),
                            stop=(eb == EB - 1),
                        )
                    rope(
                        ktc[ch][:, db * 512:(db + 1) * 512],
                        ps,
                        ck[:],
                        sk[:],
                        512,
                    )
                # V natural layout, interleaved ones column
                for stl in range(4):
                    st = ch * 4 + stl
                    psa = psb.tile([128, 512], f32, tag="ps_proj")
                    psb2 = pss.tile([128, 256], f32, tag="ps_vb")
                    for eb in range(EB):
                        nc.tensor.matmul(
                            psa[:],
                            xt_t[:, eb * 512 + stl * 128:eb * 512 + stl * 128 + 128],
                            wv_sb[:, eb * D_MODEL:eb * D_MODEL + 512],
                            start=(eb == 0),
                            stop=(eb == EB - 1),
                        )
                    for eb in range(EB):
                        nc.tensor.matmul(
                            psb2[:],
                            xt_t[:, eb * 512 + stl * 128:eb * 512 + stl * 128 + 128],
                            wv_sb[:, eb * D_MODEL + 512:eb * D_MODEL + 768],
                            start=(eb == 0),
                            stop=(eb == EB - 1),
                        )
                    base = stl * VW
                    vtile = vc[ch][:, base:base + VW].rearrange(
                        "p (h d) -> p h d", d=65
                    )
                    nc.vector.memset(vtile[:, :, 64:65], 1.0)
                    nc.vector.tensor_copy(
                        vtile[:, 0:8, 0:64],
                        psa[:].rearrange("p (h d) -> p h d", d=64),
                    )
                    nc.vector.tensor_copy(
                        vtile[:, 8:12, 0:64],
                        psb2[:].rearrange("p (h d) -> p h d", d=64),
                    )

            # ---- attention ----
            for h in range(H):
                kb = h // 2
                ro = 64 * (h % 2)
                for s in range(2):
                    n_pairs = 8 if s == 0 else 16
                    itmap = IT0_MAP if s == 0 else IT1_MAP
                    bias_off = 0 if s == 0 else 8
                    pv = psv.tile([65, QT], f32, tag="ps_pv")
                    for p in range(n_pairs):
                        i0 = itmap[2 * p]
                        i1 = itmap[2 * p + 1]
                        sc = pssc.tile([128, 512], f32, tag="ps_sc")
                        nc.tensor.matmul(
                            sc[:, 0:256],
                            ktc[i0 // 4][ro:ro + 64, kb * 512 + (i0 % 4) * 128:kb * 512 + (i0 % 4) * 128 + 128],
                            qt[ro:ro + 64, kb * 2 * QT + s * QT:kb * 2 * QT + s * QT + QT],
                            start=True,
                            stop=True,
                        )
                        nc.tensor.matmul(
                            sc[:, 256:512],
                            ktc[i1 // 4][ro:ro + 64, kb * 512 + (i1 % 4) * 128:kb * 512 + (i1 % 4) * 128 + 128],
                            qt[ro:ro + 64, kb * 2 * QT + s * QT:kb * 2 * QT + s * QT + QT],
                            start=True,
                            stop=True,
                        )
                        et = epool.tile([128, 512], bf16, tag="et")
                        nc.scalar.activation(
                            et[:],
                            sc[:],
                            Exp,
                            bias=biasp[:, bias_off + p:bias_off + p + 1],
                            scale=0.125,
                        )
                        if p == 0:
                            nc.vector.tensor_mul(et[:], et[:], maskab[:])
                        nc.tensor.matmul(
                            pv[:],
                            vc[i0 // 4][:, (i0 % 4) * VW + h * 65:(i0 % 4) * VW + h * 65 + 65],
                            et[:, 0:256],
                            start=(p == 0),
                            stop=False,
                        )
                        nc.tensor.matmul(
                            pv[:],
                            vc[i1 // 4][:, (i1 % 4) * VW + h * 65:(i1 % 4) * VW + h * 65 + 65],
                            et[:, 256:512],
                            start=False,
                            stop=(p == n_pairs - 1),
                        )
                    # normalize: recip of sums (row 64) -> physical row 0
                    # (partition_broadcast reads the tile's partition 0),
                    # broadcast, multiply
                    rc = npool.tile([128, QT], f32, tag="recip")
                    nc.vector.reciprocal(rc[64:65, :], pv[64:65, :])
                    r0 = npool.tile([1, QT], f32, tag="r0")
                    nc.sync.dma_start(out=r0[:], in_=rc[64:65, :])
                    rb = npool.tile([128, QT], f32, tag="rbcast")
                    nc.gpsimd.partition_broadcast(rb[0:64, :], r0[0:1, :])
                    nc.vector.tensor_mul(
                        attn[0:64, h * 2 * QT + s * QT:h * 2 * QT + s * QT + QT],
                        pv[0:64, :],
                        rb[0:64, :],
                    )

            # ---- output projection (contraction over heads, K=64 each) ----
            # wo resident as two half tiles reusing the freed wq/wk/wv slots
            wo_half = []
            for g in range(2):
                t = wpool.tile([64, 6 * D_MODEL], bf16, tag="w", name=f"wo{g}")
                for j in range(6):
                    h = 6 * g + j
                    nc.sync.dma_start(
                        out=t[:, j * D_MODEL:(j + 1) * D_MODEL],
                        in_=w_d["wo"][h * 64:(h + 1) * 64, :],
                    )
                wo_half.append(t)
            for qtl in range(4):
                po1 = pssc.tile([128, 512], f32, tag="ps_sc")
                po2 = pss.tile([128, 256], f32, tag="ps_vb")
                for h in range(H):
                    lhs = attn[0:64, h * 2 * QT + qtl * 128:h * 2 * QT + qtl * 128 + 128]
                    wo_t = wo_half[h // 6]
                    off = (h % 6) * D_MODEL
                    nc.tensor.matmul(
                        po1[:],
                        lhs,
                        wo_t[:, off:off + 512],
                        start=(h == 0),
                        stop=(h == H - 1),
                    )
                    nc.tensor.matmul(
                        po2[:],
                        lhs,
                        wo_t[:, off + 512:off + 768],
                        start=(h == 0),
                        stop=(h == H - 1),
                    )
                osb = opool.tile([128, D_MODEL], f32, tag="osb")
                nc.vector.tensor_copy(osb[:, 0:512], po1[:])
                nc.vector.tensor_copy(osb[:, 512:768], po2[:])
                nc.sync.dma_start(
                    out=out_d[qtl * 128:(qtl + 1) * 128, :], in_=osb[:]
                )

    nc.compile()
    return nc


_PROGRAM = None


def _get_program():
    global _PROGRAM
    if _PROGRAM is None:
        _PROGRAM = build_program()
    return _PROGRAM


def host_prep(in_features, token_positions, q_proj, k_proj, v_proj, o_proj):
    """Build the 8 per-core input maps."""
    x = np.asarray(in_features, np.float32).reshape(S, D_MODEL)
    tp = np.asarray(token_positions)
    qp = np.asarray(q_proj, np.float32)
    kp = np.asarray(k_proj, np.float32)
    vp = np.asarray(v_proj, np.float32)
    op = np.asarray(o_proj, np.float32)

    xt = np.ascontiguousarray(x.T)                      # [768, 4096] fp32
    xt_bf = xt.astype(BF16)
    wq = np.ascontiguousarray(qp.T).astype(BF16)
    wk = np.ascontiguousarray(kp.T).astype(BF16)
    wv = np.ascontiguousarray(vp.T).astype(BF16)
    wo = np.ascontiguousarray(op.T).astype(BF16)

    inv_freq = 1.0 / THETA ** (np.arange(0, DK, 2, dtype=np.float32) / DK)
    pos = np.clip(tp.astype(np.float32), 0, MAX_SEQ_LEN - 1)
    freq = pos[:, None] * inv_freq[None, :]             # [S, 32]
    cos_t, sin_t = np.cos(freq), np.sin(freq)

    r = np.arange(128)
    fidx = (r % 64) // 2
    sign = np.where(r % 2 == 0, -1.0, 1.0).astype(np.float32)
    cos128 = cos_t[:, fidx].T.astype(np.float32)        # [128, S]
    sin128 = (sin_t[:, fidx].T * sign[:, None]).astype(np.float32)

    # diagonal masks: A = k-tile aligned with q[0:256) first half,
    # B = aligned with second half. scores^T layout: [k(128), q(256)].
    ki = np.arange(128)[:, None]
    qi = np.arange(QT)[None, :]
    mask_a = (ki <= qi).astype(np.float32)
    mask_b = (ki + 128 <= qi).astype(np.float32)
    maskab = np.concatenate([mask_a, mask_b], axis=1).astype(BF16)

    in_maps = []
    for c in range(N_CORES):
        sig = _sigma(c)
        perm = np.concatenate(
            [np.arange(t * 128, (t + 1) * 128) for t in sig]
        )
        qcols = np.concatenate(
            [
                np.arange(QT * c, QT * (c + 1)),
                np.arange(QT * (8 + c), QT * (9 + c)),
            ]
        )
        biasp = np.broadcast_to(_bias_cols(c)[None, :], (128, 24))
        in_maps.append(
            {
                "xt": np.ascontiguousarray(xt_bf[:, perm]),
                "xtq": np.ascontiguousarray(xt_bf[:, qcols]),
                "wq": wq,
                "wk": wk,
                "wv": wv,
                "wo": wo,
                "cosk": np.ascontiguousarray(cos128[:, perm]).astype(BF16),
                "sink": np.ascontiguousarray(sin128[:, perm]).astype(BF16),
                "cosq": np.ascontiguousarray(cos128[:, qcols]).astype(BF16),
                "sinq": np.ascontiguousarray(sin128[:, qcols]).astype(BF16),
                "maskab": maskab,
                "biasp": np.ascontiguousarray(biasp, np.float32),
            }
        )
    return in_maps


def assemble_output(results):
    out = np.empty((1, S, D_MODEL), np.float32)
    for c in range(N_CORES):
        r = np.asarray(results[c]["out"], np.float32)
        out[0, QT * c:QT * (c + 1)] = r[0:QT]
        out[0, QT * (8 + c):QT * (9 + c)] = r[QT:2 * QT]
    return out


def kernel(**inputs):
    from concourse.bass_utils import run_bass_kernel_spmd

    nc = _get_program()
    in_maps = host_prep(**inputs)
    res = run_bass_kernel_spmd(nc, in_maps, list(range(N_CORES)))
    return assemble_output(res.results)


if __name__ == "__main__":
    nc = build_program()
    print("program built and compiled")
